# revision 18
# baseline (speedup 1.0000x reference)
"""Trainium2 Bass kernel for nn_Attention_61229053772048 (dual-softmax linear attention).

v5 design. Sharding: data-parallel over batch B=8, one batch element per core.

Structure (see v4 notes in git history of this file):
  - fp8-e4m3 DoubleRow matmuls for K/Q/V projections, the ctx accumulation
    (expK and V both stored fp8; expK scaled 1/4 to stay in e4m3 range, the
    scale cancels in the Sk normalization), and the folded output projection.
  - Host-side prep: x pre-permuted+cast to the transposed fp8 operand layout,
    weights pre-transposed/cast, residual+bias pre-added (bf16). No device
    transposes or casts of x.
  - Wp@ctx factorization: out proj = (Wp @ bdctx)^T-product applied directly
    to the normalized expQ8, removing the attended matmul and one PSUM drain.
  - Engine balance: ACT = 16 exps + 4 tail copies; DVE = V drains + 1/Sq +
    tail; Pool = expQ8 muls + 2 tail adds; sync issues all DMAs.

Index conventions (raw-reshape semantics of the reference):
  token n in [0,4096); attention channel d' = n>>4 (=r), position
  pos = (n&15)*256 + o where o is the projection output channel.
  x8[p, t, q, r] = x[16r+q, 128t+p]    (fp8, host-prepped)
  w8[p, t, o]    = W[o, 128t+p]        (fp8/bf16, host-prepped)
  xres[o2, pos]  = x.reshape(256,4096) + bp[:,None]   (bf16, host-prepped)
  out y[o2, pos] -> host reshape (4096, 256) f32.
"""

import sys

sys.path.insert(0, "/opt/trn_rl_repo")

import ml_dtypes
import numpy as np

import concourse.bass as bass
import concourse.bacc as bacc_mod
import concourse.tile as tile
from concourse import mybir
from concourse.bass_utils import run_bass_kernel_spmd
from concourse.masks import make_identity

F32 = mybir.dt.float32
BF16 = mybir.dt.bfloat16
F8 = mybir.dt.float8e4
Exp = mybir.ActivationFunctionType.Exp
Copy = mybir.ActivationFunctionType.Copy
DR = mybir.MatmulPerfMode.DoubleRow
Mult = mybir.AluOpType.mult
Add = mybir.AluOpType.add
Div = mybir.AluOpType.divide

NP_F8 = ml_dtypes.float8_e4m3
NP_BF16 = ml_dtypes.bfloat16

N, C, P = 4096, 256, 128
NCORES = 8
SCALE_Q = 32.0
SCALE_P = 32.0
INV_S = 1.0 / (SCALE_Q * SCALE_P)
NEG_LN4 = -1.3862943611198906  # expK8 = exp(K - ln4) = exp(K)/4, cancels in Sk

_CACHE = {}


def _build_program():
    nc = bacc_mod.Bacc(None, target_bir_lowering=False, debug=False)
    x8_e = nc.declare_dram_parameter("x8", [P, 2, 16, 256], F8, isOutput=False)
    wk_e = nc.declare_dram_parameter("wk8", [P, 2, C], F8, isOutput=False)
    wv_e = nc.declare_dram_parameter("wv8", [P, 2, C], F8, isOutput=False)
    wq_e = nc.declare_dram_parameter("wq8", [P, 2, C], F8, isOutput=False)
    wp_e = nc.declare_dram_parameter("wpT", [P, 2, C], BF16, isOutput=False)
    xr_e = nc.declare_dram_parameter("xres", [C, N], BF16, isOutput=False)
    y_e = nc.declare_dram_parameter("y", [C, N], BF16, isOutput=True)

    with tile.TileContext(nc) as tc:
        _body(tc, x8_e, wk_e, wv_e, wq_e, wp_e, xr_e, y_e)
    nc.compile()
    return nc


def _body(tc, x8_e, wk_e, wv_e, wq_e, wp_e, xr_e, y_e):
    nc = tc.nc
    from contextlib import ExitStack

    ctx = ExitStack()
    consts = ctx.enter_context(tc.tile_pool(name="consts", bufs=1))
    bigs = ctx.enter_context(tc.tile_pool(name="bigs", bufs=1))
    ypool = ctx.enter_context(tc.tile_pool(name="y", bufs=4))
    pmm = ctx.enter_context(tc.tile_pool(name="pmm", bufs=1, space="PSUM"))
    pctx = ctx.enter_context(tc.tile_pool(name="pctx", bufs=1, space="PSUM"))

    # ---- SBUF buffers ----
    x8 = bigs.tile([P, 2, 16, 256], F8, tag="x8")
    wk8 = bigs.tile([P, 2, C], F8, tag="wk8")
    wv8 = bigs.tile([P, 2, C], F8, tag="wv8")
    wq8 = bigs.tile([P, 2, C], F8, tag="wq8")
    wpT = bigs.tile([P, 2, C], BF16, tag="wpT")
    expK8 = bigs.tile([P, 8, 2, 2, 256], F8, tag="expK8")  # (qp, oc, qi, r)
    Vb = bigs.tile([P, 8, 2, 2, 2, 130], F8, tag="Vb")  # (qp, qi, g, oc, e|1|0)
    expQall = bigs.tile([P, 8, 2, 2, 256], BF16, tag="expQall")  # (qp, g, qi, o)
    expQ8 = bigs.tile([P, 8, 2, 512], F8, tag="expQ8")
    rsq = bigs.tile([P, 8, 2, 512], F32, tag="rsq")
    xres = bigs.tile([P, 2, N], BF16, tag="xres")
    rsk = bigs.tile([P, 2], F32, tag="rsk")
    ctx_sbb = bigs.tile([P, 2, P], BF16, tag="ctx_sbb")
    bdctxT = bigs.tile([P, 2, P], BF16, tag="bdctxT")
    wpc8 = bigs.tile([P, 2, C], F8, tag="wpc8")
    warm = bigs.tile([P, 1], BF16, tag="warm")

    # ---- constants / init ----
    identity = consts.tile([P, P], BF16)
    make_identity(nc, identity)
    # block-diag 32x32 of 1/32: Sq matmul yields Sq/32, so recip = 32/Sq and
    # the fp8 quantization of expQ8 = expQ * (32/Sq) is centered around ~1.
    blockones = consts.tile([P, P], BF16)
    nc.vector.memset(blockones, 0.0)
    for k in range(4):
        nc.vector.memset(
            blockones[32 * k : 32 * k + 32, 32 * k : 32 * k + 32], 1.0 / SCALE_Q
        )
    nc.vector.memset(Vb[:, :, :, :, :, 128:129], 1.0)
    nc.vector.memset(Vb[:, :, :, :, :, 129:130], 0.0)
    nc.vector.memset(bdctxT, 0.0)
    negln4 = consts.tile([P, 1], F32)
    nc.vector.memset(negln4, NEG_LN4)

    # ---- input DMAs: everything on the sync queue (idle otherwise) except
    # wv8/wq8 on the vector queue so V-mm(0)/Q-mm(0) aren't gated by the
    # serial sync issue stream. Out DMAs also go to sync (tail). ----
    nc.sync.dma_start(out=x8[:, :, 0:4, :], in_=x8_e[:, :, 0:4, :])
    nc.sync.dma_start(out=wk8, in_=wk_e.rearrange("p t c -> p t c"))
    nc.scalar.dma_start(out=wv8, in_=wv_e.rearrange("p t c -> p t c"))
    nc.scalar.dma_start(out=wq8, in_=wq_e.rearrange("p t c -> p t c"))
    # preload the ACT exp table during startup so it doesn't stall K-exp(0)
    nc.scalar.activation(out=warm, in_=identity[:, 0:1], func=Exp)
    nc.sync.dma_start(out=x8[:, :, 4:8, :], in_=x8_e[:, :, 4:8, :])
    nc.sync.dma_start(out=x8[:, :, 8:12, :], in_=x8_e[:, :, 8:12, :])
    nc.sync.dma_start(out=x8[:, :, 12:16, :], in_=x8_e[:, :, 12:16, :])
    nc.sync.dma_start(out=wpT, in_=wp_e.rearrange("p t c -> p t c"))
    xr_v = xr_e.rearrange("(cc p) n -> p cc n", p=P)
    for h in range(2):
        nc.sync.dma_start(
            out=xres[:, :, 2048 * h : 2048 * h + 2048],
            in_=xr_v[:, :, 2048 * h : 2048 * h + 2048],
        )

    # ---- persistent ctx accumulator: both g halves in ONE psum bank.
    # Only the very first matmul into the bank uses start=True (zero-region
    # granularity is the whole 2KB bank); every other first-write relies on
    # the pending-zero semantics.
    ctxp = pctx.tile([P, 2, 132], F32, tag="ctxp", name="ctxp")

    def emit_lagged(qp):
        """ctx-DR + Sq + recip + mul for chunk qp (PE/DVE/Pool)."""
        sqs = []

        def ctx_mm(qi, g):
            nc.tensor.matmul(
                ctxp[:, g, 0:130],
                lhsT=expK8[:, qp, :, qi, 128 * g : 128 * g + 128],
                rhs=Vb[:, qp, qi, g, :, :],
                start=(qp == 0 and qi == 0 and g == 0),
                stop=(qp == 7 and qi == 1),
                perf_mode=DR,
                skip_group_check=True,
            )

        ctx_mm(0, 0)
        for g in range(2):
            sq_g = pmm.tile([P, 512], F32, tag="SQ", name=f"sq{g}")
            nc.tensor.matmul(
                sq_g,
                lhsT=blockones,
                rhs=expQall[:, qp, g].rearrange("p a b -> p (a b)"),
                start=True,
                stop=True,
            )
            sqs.append(sq_g)
            if g == 0:
                ctx_mm(0, 1)
                ctx_mm(1, 0)
            else:
                ctx_mm(1, 1)
        for g in range(2):
            nc.vector.reciprocal_approx_fast(out=rsq[:, qp, g], in_=sqs[g])
        nc.gpsimd.tensor_tensor(
            out=expQ8[:, qp],
            in0=expQall[:, qp].rearrange("p g qi o -> p g (qi o)"),
            in1=rsq[:, qp],
            op=Mult,
        )

    # ---- main loop. Per-qp PE order: K, lagged(qp-1) ctx/Sq, V, Q — the
    # V matmuls cover the DVE recip latency so the Q matmuls (which reuse
    # the Sq psum slot) never stall. ----
    for qp in range(8):
        kps = pmm.tile([P, 4, 256], F32, tag="K", name="kps")

        def kv_mm(w8, ps, oc):
            nc.tensor.matmul(
                ps[:, 2 * oc : 2 * oc + 2, :],
                lhsT=w8[:, :, 128 * oc : 128 * oc + 128],
                rhs=x8[:, :, 2 * qp : 2 * qp + 2, :],
                start=True,
                stop=True,
                perf_mode=DR,
            )

        kv_mm(wk8, kps, 0)
        kv_mm(wk8, kps, 1)
        nc.scalar.activation(
            out=expK8[:, qp].rearrange("p a b c -> p (a b c)"),
            in_=kps.rearrange("p a b -> p (a b)"),
            func=Exp,
            bias=negln4,
        )
        if qp >= 1:
            emit_lagged(qp - 1)
        vps = pmm.tile([P, 4, 256], F32, tag="V", name="vps")
        kv_mm(wv8, vps, 0)
        kv_mm(wv8, vps, 1)
        nc.vector.tensor_copy(
            out=Vb[:, qp, :, :, :, 0:128],
            in_=vps.rearrange("p (oc qi) (g e) -> p qi g oc e", oc=2, g=2),
        )
        qps_t = pmm.tile([P, 4, 256], F32, tag="QS", name="qps")
        for rc in range(2):
            for qi in range(2):
                nc.tensor.matmul(
                    qps_t[:, 2 * rc + qi, :],
                    lhsT=x8[:, :, 2 * qp + qi, 128 * rc : 128 * rc + 128],
                    rhs=wq8,
                    start=(qi == 0),
                    stop=(qi == 1),
                    perf_mode=DR,
                    skip_group_check=True,
                )
        nc.scalar.activation(
            out=expQall[:, qp].rearrange("p a b c -> p (a b c)"),
            in_=qps_t.rearrange("p a b -> p (a b)"),
            func=Exp,
        )
    emit_lagged(7)

    # ---- barrier: normalize ctx, build bdctxT, fold Wp ----
    for g in range(2):
        nc.vector.reciprocal_approx_fast(
            out=rsk[:, g : g + 1], in_=ctxp[:, g, 128:129]
        )
        nc.vector.tensor_scalar_mul(
            out=ctx_sbb[:, g, :], in0=ctxp[:, g, 0:128], scalar1=rsk[:, g : g + 1]
        )
    tps = []
    for g in range(2):
        tp = pmm.tile([P, P], BF16, tag="K" if g == 0 else "V", name=f"tp{g}")
        nc.tensor.transpose(tp, ctx_sbb[:, g, :], identity)
        tps.append(tp)
    for g in range(2):
        for k in range(4):
            s = slice(32 * k, 32 * k + 32)
            nc.vector.tensor_copy(out=bdctxT[s, g, s], in_=tps[g][s, s])
    wcps = pmm.tile([P, 2, C], F32, tag="SQ", name="wcps")
    for g in range(2):
        nc.tensor.matmul(
            wcps[:, g, :],
            lhsT=bdctxT[:, g, :],
            rhs=wpT[:, g, :],
            start=(g == 0),
            stop=(g == 1),
            skip_group_check=True,
        )
    nc.scalar.activation(
        out=wpc8.rearrange("p a b -> p (a b)"),
        in_=wcps.rearrange("p a b -> p (a b)"),
        func=Copy,
        scale=SCALE_P,
    )
    # keep the PE busy across the barrier so the clock doesn't downshift
    # before the 16 output-projection matmuls
    for w in range(2):
        wmm = pmm.tile([P, 2, 512], F32, tag="V" if w == 0 else "K", name="wmm")
        nc.tensor.matmul(
            wmm[:, 0, :],
            lhsT=blockones,
            rhs=expQall[:, 0, 0].rearrange("p a b -> p (a b)"),
            start=True,
            stop=True,
        )

    # ---- tail: proj + drain + residual + out DMA ----
    y_v = y_e.rearrange("(cc p) n -> p cc n", p=P)
    for qq in range(8):
        pps = pmm.tile([P, 2, 512], F32, tag="K" if qq % 2 == 0 else "V", name="pps")
        for oc in range(2):
            nc.tensor.matmul(
                pps[:, oc, :],
                lhsT=wpc8[:, :, 128 * oc : 128 * oc + 128],
                rhs=expQ8[:, qq],
                start=True,
                stop=True,
                perf_mode=DR,
            )
        yt = ypool.tile([P, 2, 512], BF16, tag="yt", name="yt")
        if qq % 2 == 0:
            nc.vector.scalar_tensor_tensor(
                out=yt,
                in0=pps,
                scalar=INV_S,
                in1=xres[:, :, 512 * qq : 512 * qq + 512],
                op0=Mult,
                op1=Add,
            )
        else:
            ys = ypool.tile([P, 2, 512], BF16, tag="ys", name="ys")
            nc.scalar.activation(out=ys, in_=pps, func=Copy, scale=INV_S)
            nc.vector.tensor_tensor(
                out=yt, in0=ys, in1=xres[:, :, 512 * qq : 512 * qq + 512], op=Add
            )
        nc.sync.dma_start(out=y_v[:, :, 512 * qq : 512 * qq + 512], in_=yt)

    ctx.close()


def _get_nc():
    if "nc" not in _CACHE:
        _CACHE["nc"] = _build_program()
    return _CACHE["nc"]


def _prep_core(x, wk8, wv8, wq8, wpT, bp):
    """Host-side layout prep for one batch element."""
    xq = x.reshape(256, 16, 256)  # [r, q, c]
    x8 = np.ascontiguousarray(
        xq.transpose(2, 1, 0).reshape(2, 128, 16, 256).transpose(1, 0, 2, 3)
    ).astype(NP_F8)  # [p, t, q, r]
    xres = (x.reshape(256, 4096) + bp[:, None]).astype(NP_BF16)
    return dict(x8=x8, wk8=wk8, wv8=wv8, wq8=wq8, wpT=wpT, xres=xres)


def _prep_w(W, dt):
    return np.ascontiguousarray(W.T.reshape(2, 128, 256).transpose(1, 0, 2)).astype(dt)


def kernel(**inputs):
    x = np.ascontiguousarray(np.asarray(inputs["x"], dtype=np.float32))
    B = x.shape[0]
    bp = np.asarray(inputs["bp"], dtype=np.float32)
    wk8 = _prep_w(np.asarray(inputs["Wk"], dtype=np.float32), NP_F8)
    wv8 = _prep_w(np.asarray(inputs["Wv"], dtype=np.float32), NP_F8)
    wq8 = _prep_w(np.asarray(inputs["Wq"], dtype=np.float32), NP_F8)
    wpT = _prep_w(np.asarray(inputs["Wp"], dtype=np.float32), NP_BF16)

    nc = _get_nc()
    in_maps = [_prep_core(x[b], wk8, wv8, wq8, wpT, bp) for b in range(B)]
    res = run_bass_kernel_spmd(nc, in_maps, list(range(NCORES)))
    out = np.stack(
        [
            np.asarray(res.results[b]["y"])
            .astype(np.float32)
            .reshape(4096, 256)
            for b in range(B)
        ],
        axis=0,
    )
    return out


# revision 19
# speedup vs baseline: 1.1689x; 1.1689x over previous
"""Trainium2 Bass kernel for nn_Attention_61229053772048 (dual-softmax linear attention).

v5 design. Sharding: data-parallel over batch B=8, one batch element per core.

Structure (see v4 notes in git history of this file):
  - fp8-e4m3 DoubleRow matmuls for K/Q/V projections, the ctx accumulation
    (expK and V both stored fp8; expK scaled 1/4 to stay in e4m3 range, the
    scale cancels in the Sk normalization), and the folded output projection.
  - Host-side prep: x pre-permuted+cast to the transposed fp8 operand layout,
    weights pre-transposed/cast, residual+bias pre-added (bf16). No device
    transposes or casts of x.
  - Wp@ctx factorization: out proj = (Wp @ bdctx)^T-product applied directly
    to the normalized expQ8, removing the attended matmul and one PSUM drain.
  - Engine balance: ACT = 16 exps + 4 tail copies; DVE = V drains + 1/Sq +
    tail; Pool = expQ8 muls + 2 tail adds; sync issues all DMAs.

Index conventions (raw-reshape semantics of the reference):
  token n in [0,4096); attention channel d' = n>>4 (=r), position
  pos = (n&15)*256 + o where o is the projection output channel.
  x8[p, t, q, r] = x[16r+q, 128t+p]    (fp8, host-prepped)
  w8[p, t, o]    = W[o, 128t+p]        (fp8/bf16, host-prepped)
  xres[o2, pos]  = x.reshape(256,4096) + bp[:,None]   (bf16, host-prepped)
  out y[o2, pos] -> host reshape (4096, 256) f32.
"""

import sys

sys.path.insert(0, "/opt/trn_rl_repo")

import ml_dtypes
import numpy as np

import concourse.bass as bass
import concourse.bacc as bacc_mod
import concourse.tile as tile
from concourse import mybir
from concourse.bass_utils import run_bass_kernel_spmd
from concourse.masks import make_identity

F32 = mybir.dt.float32
BF16 = mybir.dt.bfloat16
F8 = mybir.dt.float8e4
Exp = mybir.ActivationFunctionType.Exp
Copy = mybir.ActivationFunctionType.Copy
DR = mybir.MatmulPerfMode.DoubleRow
Mult = mybir.AluOpType.mult
Add = mybir.AluOpType.add
Div = mybir.AluOpType.divide

NP_F8 = ml_dtypes.float8_e4m3
NP_BF16 = ml_dtypes.bfloat16

N, C, P = 4096, 256, 128
NCORES = 8
SCALE_Q = 32.0
SCALE_P = 32.0
INV_S = 1.0 / (SCALE_Q * SCALE_P)
NEG_LN4 = -1.3862943611198906  # expK8 = exp(K - ln4) = exp(K)/4, cancels in Sk

_CACHE = {}


def _build_program():
    nc = bacc_mod.Bacc(None, target_bir_lowering=False, debug=False)
    x8_e = nc.declare_dram_parameter("x8", [P, 2, 16, 256], F8, isOutput=False)
    wk_e = nc.declare_dram_parameter("wk8", [P, 2, C], F8, isOutput=False)
    wv_e = nc.declare_dram_parameter("wv8", [P, 2, C], F8, isOutput=False)
    wq_e = nc.declare_dram_parameter("wq8", [P, 2, C], F8, isOutput=False)
    wp_e = nc.declare_dram_parameter("wpT", [P, 2, C], BF16, isOutput=False)
    xr_e = nc.declare_dram_parameter("xres", [C, N], BF16, isOutput=False)
    y_e = nc.declare_dram_parameter("y", [C, N], BF16, isOutput=True)

    with tile.TileContext(nc) as tc:
        _body(tc, x8_e, wk_e, wv_e, wq_e, wp_e, xr_e, y_e)
    nc.compile()
    return nc


def _body(tc, x8_e, wk_e, wv_e, wq_e, wp_e, xr_e, y_e):
    nc = tc.nc
    from contextlib import ExitStack

    ctx = ExitStack()
    consts = ctx.enter_context(tc.tile_pool(name="consts", bufs=1))
    bigs = ctx.enter_context(tc.tile_pool(name="bigs", bufs=1))
    ypool = ctx.enter_context(tc.tile_pool(name="y", bufs=4))
    pmm = ctx.enter_context(tc.tile_pool(name="pmm", bufs=1, space="PSUM"))
    pctx = ctx.enter_context(tc.tile_pool(name="pctx", bufs=1, space="PSUM"))

    # ---- SBUF buffers ----
    x8 = bigs.tile([P, 2, 16, 256], F8, tag="x8")
    wk8 = bigs.tile([P, 2, C], F8, tag="wk8")
    wv8 = bigs.tile([P, 2, C], F8, tag="wv8")
    wq8 = bigs.tile([P, 2, C], F8, tag="wq8")
    wpT = bigs.tile([P, 2, C], BF16, tag="wpT")
    expK8 = bigs.tile([P, 8, 2, 2, 256], F8, tag="expK8")  # (qp, oc, qi, r)
    Vb = bigs.tile([P, 8, 2, 2, 2, 130], F8, tag="Vb")  # (qp, qi, g, oc, e|1|0)
    expQall = bigs.tile([P, 8, 2, 2, 256], BF16, tag="expQall")  # (qp, g, qi, o)
    expQ8 = bigs.tile([P, 8, 2, 512], F8, tag="expQ8")
    rsq = bigs.tile([P, 8, 2, 512], F32, tag="rsq")
    xres = bigs.tile([P, 2, N], BF16, tag="xres")
    rsk = bigs.tile([P, 2], F32, tag="rsk")
    ctx_sbb = bigs.tile([P, 2, P], BF16, tag="ctx_sbb")
    bdctxT = bigs.tile([P, 2, P], BF16, tag="bdctxT")
    wpc8 = bigs.tile([P, 2, C], F8, tag="wpc8")
    warm = bigs.tile([P, 1], BF16, tag="warm")

    # ---- constants / init ----
    identity = consts.tile([P, P], BF16)
    make_identity(nc, identity)
    # block-diag 32x32 of 1/32: Sq matmul yields Sq/32, so recip = 32/Sq and
    # the fp8 quantization of expQ8 = expQ * (32/Sq) is centered around ~1.
    blockones = consts.tile([P, P], BF16)
    nc.vector.memset(blockones, 0.0)
    for k in range(4):
        nc.vector.memset(
            blockones[32 * k : 32 * k + 32, 32 * k : 32 * k + 32], 1.0 / SCALE_Q
        )
    nc.vector.memset(Vb[:, :, :, :, :, 128:129], 1.0)
    nc.vector.memset(Vb[:, :, :, :, :, 129:130], 0.0)
    nc.vector.memset(bdctxT, 0.0)
    negln4 = consts.tile([P, 1], F32)
    nc.vector.memset(negln4, NEG_LN4)

    # ---- input DMAs: everything on the sync queue (idle otherwise) except
    # wv8/wq8 on the vector queue so V-mm(0)/Q-mm(0) aren't gated by the
    # serial sync issue stream. Out DMAs also go to sync (tail). ----
    nc.sync.dma_start(out=x8[:, :, 0:4, :], in_=x8_e[:, :, 0:4, :])
    nc.sync.dma_start(out=wk8, in_=wk_e.rearrange("p t c -> p t c"))
    nc.scalar.dma_start(out=wv8, in_=wv_e.rearrange("p t c -> p t c"))
    nc.scalar.dma_start(out=wq8, in_=wq_e.rearrange("p t c -> p t c"))
    # preload the ACT exp table during startup so it doesn't stall K-exp(0)
    nc.scalar.activation(out=warm, in_=identity[:, 0:1], func=Exp)
    nc.sync.dma_start(out=x8[:, :, 4:8, :], in_=x8_e[:, :, 4:8, :])
    nc.sync.dma_start(out=x8[:, :, 8:12, :], in_=x8_e[:, :, 8:12, :])
    nc.sync.dma_start(out=x8[:, :, 12:16, :], in_=x8_e[:, :, 12:16, :])
    nc.sync.dma_start(out=wpT, in_=wp_e.rearrange("p t c -> p t c"))
    xr_v = xr_e.rearrange("(cc p) n -> p cc n", p=P)
    for h in range(2):
        nc.sync.dma_start(
            out=xres[:, :, 2048 * h : 2048 * h + 2048],
            in_=xr_v[:, :, 2048 * h : 2048 * h + 2048],
        )

    # ---- persistent ctx accumulator: both g halves in ONE psum bank.
    # Only the very first matmul into the bank uses start=True (zero-region
    # granularity is the whole 2KB bank); every other first-write relies on
    # the pending-zero semantics.
    ctxp = pctx.tile([P, 2, 132], F32, tag="ctxp", name="ctxp")

    def emit_lagged(qp):
        """ctx-DR + Sq + recip + mul for chunk qp (PE/DVE/Pool)."""
        sqs = []

        def ctx_mm(qi, g):
            nc.tensor.matmul(
                ctxp[:, g, 0:130],
                lhsT=expK8[:, qp, :, qi, 128 * g : 128 * g + 128],
                rhs=Vb[:, qp, qi, g, :, :],
                start=(qp == 0 and qi == 0 and g == 0),
                stop=(qp == 7 and qi == 1),
                perf_mode=DR,
                skip_group_check=True,
            )

        ctx_mm(0, 0)
        for g in range(2):
            sq_g = pmm.tile([P, 512], F32, tag="SQ", name=f"sq{g}")
            nc.tensor.matmul(
                sq_g,
                lhsT=blockones,
                rhs=expQall[:, qp, g].rearrange("p a b -> p (a b)"),
                start=True,
                stop=True,
            )
            sqs.append(sq_g)
            if g == 0:
                ctx_mm(0, 1)
                ctx_mm(1, 0)
            else:
                ctx_mm(1, 1)
        for g in range(2):
            nc.vector.reciprocal_approx_fast(out=rsq[:, qp, g], in_=sqs[g])
        mul_eng = nc.vector if qp >= 5 else nc.gpsimd
        mul_eng.tensor_tensor(
            out=expQ8[:, qp],
            in0=expQall[:, qp].rearrange("p g qi o -> p g (qi o)"),
            in1=rsq[:, qp],
            op=Mult,
        )

    # ---- main loop. Per-qp PE order: K, lagged(qp-1) ctx/Sq, V, Q — the
    # V matmuls cover the DVE recip latency so the Q matmuls (which reuse
    # the Sq psum slot) never stall. ----
    for qp in range(8):
        kps = pmm.tile([P, 4, 256], F32, tag="K", name="kps")

        def kv_mm(w8, ps, oc):
            nc.tensor.matmul(
                ps[:, 2 * oc : 2 * oc + 2, :],
                lhsT=w8[:, :, 128 * oc : 128 * oc + 128],
                rhs=x8[:, :, 2 * qp : 2 * qp + 2, :],
                start=True,
                stop=True,
                perf_mode=DR,
            )

        kv_mm(wk8, kps, 0)
        kv_mm(wk8, kps, 1)
        nc.scalar.activation(
            out=expK8[:, qp].rearrange("p a b c -> p (a b c)"),
            in_=kps.rearrange("p a b -> p (a b)"),
            func=Exp,
            bias=negln4,
        )
        if qp >= 1:
            emit_lagged(qp - 1)
        vps = pmm.tile([P, 4, 256], F32, tag="V", name="vps")
        kv_mm(wv8, vps, 0)
        kv_mm(wv8, vps, 1)
        nc.vector.tensor_copy(
            out=Vb[:, qp, :, :, :, 0:128],
            in_=vps.rearrange("p (oc qi) (g e) -> p qi g oc e", oc=2, g=2),
        )
        qps_t = pmm.tile([P, 4, 256], F32, tag="QS", name="qps")
        for rc in range(2):
            for qi in range(2):
                nc.tensor.matmul(
                    qps_t[:, 2 * rc + qi, :],
                    lhsT=x8[:, :, 2 * qp + qi, 128 * rc : 128 * rc + 128],
                    rhs=wq8,
                    start=(qi == 0),
                    stop=(qi == 1),
                    perf_mode=DR,
                    skip_group_check=True,
                )
        nc.scalar.activation(
            out=expQall[:, qp].rearrange("p a b c -> p (a b c)"),
            in_=qps_t.rearrange("p a b -> p (a b)"),
            func=Exp,
        )
    emit_lagged(7)

    # ---- barrier: normalize ctx, build bdctxT, fold Wp ----
    for g in range(2):
        nc.vector.reciprocal_approx_fast(
            out=rsk[:, g : g + 1], in_=ctxp[:, g, 128:129]
        )
        nc.vector.tensor_scalar_mul(
            out=ctx_sbb[:, g, :], in0=ctxp[:, g, 0:128], scalar1=rsk[:, g : g + 1]
        )
    tps = []
    for g in range(2):
        tp = pmm.tile([P, P], BF16, tag="K" if g == 0 else "V", name=f"tp{g}")
        nc.tensor.transpose(tp, ctx_sbb[:, g, :], identity)
        tps.append(tp)
    for g in range(2):
        for k in range(4):
            s = slice(32 * k, 32 * k + 32)
            nc.vector.tensor_copy(out=bdctxT[s, g, s], in_=tps[g][s, s])
    wcps = pmm.tile([P, 2, C], F32, tag="SQ", name="wcps")
    for g in range(2):
        nc.tensor.matmul(
            wcps[:, g, :],
            lhsT=bdctxT[:, g, :],
            rhs=wpT[:, g, :],
            start=(g == 0),
            stop=(g == 1),
            skip_group_check=True,
        )
    nc.scalar.activation(
        out=wpc8.rearrange("p a b -> p (a b)"),
        in_=wcps.rearrange("p a b -> p (a b)"),
        func=Copy,
        scale=SCALE_P,
    )
    # keep the PE busy across the barrier so the clock doesn't downshift
    # before the 16 output-projection matmuls
    for w in range(2):
        wmm = pmm.tile([P, 2, 512], F32, tag="V" if w == 0 else "K", name="wmm")
        nc.tensor.matmul(
            wmm[:, 0, :],
            lhsT=blockones,
            rhs=expQall[:, 0, 0].rearrange("p a b -> p (a b)"),
            start=True,
            stop=True,
        )

    # ---- tail: proj + drain + residual + out DMA ----
    y_v = y_e.rearrange("(cc p) n -> p cc n", p=P)
    for qq in range(8):
        pps = pmm.tile([P, 2, 512], F32, tag=("K", "V", "QS")[qq % 3], name="pps")
        for oc in range(2):
            nc.tensor.matmul(
                pps[:, oc, :],
                lhsT=wpc8[:, :, 128 * oc : 128 * oc + 128],
                rhs=expQ8[:, qq],
                start=True,
                stop=True,
                perf_mode=DR,
            )
        yt = ypool.tile([P, 2, 512], BF16, tag="yt", name="yt")
        if qq % 2 == 0:
            nc.vector.scalar_tensor_tensor(
                out=yt,
                in0=pps,
                scalar=INV_S,
                in1=xres[:, :, 512 * qq : 512 * qq + 512],
                op0=Mult,
                op1=Add,
            )
        else:
            ys = ypool.tile([P, 2, 512], BF16, tag="ys", name="ys")
            nc.scalar.activation(out=ys, in_=pps, func=Copy, scale=INV_S)
            nc.vector.tensor_tensor(
                out=yt, in0=ys, in1=xres[:, :, 512 * qq : 512 * qq + 512], op=Add
            )
        nc.sync.dma_start(out=y_v[:, :, 512 * qq : 512 * qq + 512], in_=yt)

    ctx.close()


def _get_nc():
    if "nc" not in _CACHE:
        _CACHE["nc"] = _build_program()
    return _CACHE["nc"]


def _prep_core(x, wk8, wv8, wq8, wpT, bp):
    """Host-side layout prep for one batch element."""
    xq = x.reshape(256, 16, 256)  # [r, q, c]
    x8 = np.ascontiguousarray(
        xq.transpose(2, 1, 0).reshape(2, 128, 16, 256).transpose(1, 0, 2, 3)
    ).astype(NP_F8)  # [p, t, q, r]
    xres = (x.reshape(256, 4096) + bp[:, None]).astype(NP_BF16)
    return dict(x8=x8, wk8=wk8, wv8=wv8, wq8=wq8, wpT=wpT, xres=xres)


def _prep_w(W, dt):
    return np.ascontiguousarray(W.T.reshape(2, 128, 256).transpose(1, 0, 2)).astype(dt)


def kernel(**inputs):
    x = np.ascontiguousarray(np.asarray(inputs["x"], dtype=np.float32))
    B = x.shape[0]
    bp = np.asarray(inputs["bp"], dtype=np.float32)
    wk8 = _prep_w(np.asarray(inputs["Wk"], dtype=np.float32), NP_F8)
    wv8 = _prep_w(np.asarray(inputs["Wv"], dtype=np.float32), NP_F8)
    wq8 = _prep_w(np.asarray(inputs["Wq"], dtype=np.float32), NP_F8)
    wpT = _prep_w(np.asarray(inputs["Wp"], dtype=np.float32), NP_BF16)

    nc = _get_nc()
    in_maps = [_prep_core(x[b], wk8, wv8, wq8, wpT, bp) for b in range(B)]
    res = run_bass_kernel_spmd(nc, in_maps, list(range(NCORES)))
    out = np.stack(
        [
            np.asarray(res.results[b]["y"])
            .astype(np.float32)
            .reshape(4096, 256)
            for b in range(B)
        ],
        axis=0,
    )
    return out
